# revision 19
# baseline (speedup 1.0000x reference)
"""Trainium2 Bass kernel for nn_SelfAttentionCustom (S=2048, B=2, D=2048, H=16).

Sharding: 8 cores = 2 batches x 4 head-groups (4 heads each). Each core
computes its batch's QKV projection restricted to its heads, QK-layernorm,
RoPE, causal SDPA, and a partial output projection (wo restricted to its
heads' columns). Host sums the 4 partials per batch and adds wo_b.

All matmuls run as float32r (full PE rate at free-dim>=256, ~2e-4 rel err).
Softmax runs without max-subtraction (QK-layernorm bounds logits to ~+-12,
safe in fp32): scoresT tiles [sk,sq] -> exp -> AV matmul in transposed form
(out^T accumulated over sk tiles) and denominator row via ones-vector
matmul; reciprocal is broadcast across partitions with a K=1 matmul.
"""

import sys

sys.path.insert(0, "/opt/trn_rl_repo")

import numpy as np

import concourse.bass as bass
import concourse.tile as tile
from concourse import bacc, mybir
from concourse import bass_utils
from concourse.bass import ds, ts
from concourse.masks import make_identity

F32 = mybir.dt.float32
F32R = mybir.dt.float32r
AF = mybir.ActivationFunctionType
ALU = mybir.AluOpType

S, B, D, H = 2048, 2, 2048, 16
HD = 128          # head dim
NC = 8            # cores
HPC = 4           # heads per core
EPS = 1e-5
P = 128
ST = S // P       # 16 s-tiles
KT = D // P       # 16 contraction k-tiles (projection)
EW = HPC * HD     # 512: per-core width of q/k/v chunk
CH = 4            # sq chunks of 512
CW = S // CH      # 512 chunk width
INV_SQRT_HD = 1.0 / np.sqrt(np.float32(HD))


def _build(with_bias, qn_w_on, qn_b_on, kn_w_on, kn_b_on, **tune):
    """Build the SPMD Bass program (identical on all 8 cores)."""
    nc = bacc.Bacc("TRN2", target_bir_lowering=False, debug=False, num_devices=NC)

    kt = KT + 1 if with_bias else KT
    dk = kt * P

    xt_d = nc.dram_tensor("xt", [ST, P, kt, P], F32R, kind="ExternalInput").ap()
    wqkvt_d = nc.dram_tensor("wqkvt", [dk, 3 * EW], F32R, kind="ExternalInput").ap()
    wot_d = nc.dram_tensor("wot", [EW, D], F32R, kind="ExternalInput").ap()
    cos_d = nc.dram_tensor("cosf", [P, ST, HD // 2], F32, kind="ExternalInput").ap()
    sin_d = nc.dram_tensor("sinf", [P, ST, HD // 2], F32, kind="ExternalInput").ap()
    out_d = nc.dram_tensor("out", [S, D], F32, kind="ExternalOutput").ap()
    affine_d = {}
    for name, on in (("qnw", qn_w_on), ("qnb", qn_b_on),
                     ("knw", kn_w_on), ("knb", kn_b_on)):
        if on:
            affine_d[name] = nc.dram_tensor(
                name, [P, HD], F32, kind="ExternalInput").ap()

    reps = tune.pop("reps", 1)
    with tile.TileContext(nc) as tc:
        chain = {"last": None}
        for _ in range(reps):
            chain.pop("first_of_rep", None)
            _emit(tc, nc, kt, xt_d, wqkvt_d, wot_d, cos_d, sin_d, out_d,
                  affine_d, chain=chain, **tune)
            chain["last"] = chain.get("last_of_rep")
    nc.compile()
    return nc


def _emit(tc, nc, kt, xt_d, wqkvt_d, wot_d, cos_d, sin_d, out_d, affine_d,
          phases="ABC", psA_bufs=4, psT_bufs=4, psS_bufs=3, expp_bufs=6,
          xtp_bufs=2, psW_bufs=4, outp_bufs=6, psO_bufs=2, psD_bufs=2,
          psB_bufs=1, xt_halves=1, dma_bcast=0, tr_act=1, rope_gp=1,
          lnrope_bufs=2, chain=None, stats_bn=1, trim=1):
    from contextlib import ExitStack

    with ExitStack() as ctx:
        persist = ctx.enter_context(tc.tile_pool(name="persist", bufs=1))

        # persistent arrays
        qT = persist.tile([P, HPC, S], F32R)      # [d, h, s]
        kTt = persist.tile([P, HPC, S], F32R)
        cos_sb = persist.tile([P, ST, HD // 2], F32)
        sin_sb = persist.tile([P, ST, HD // 2], F32)
        ident_f = persist.tile([P, P], F32)
        make_identity(nc, ident_f)
        ident = persist.tile([P, P], F32R)
        nc.vector.tensor_copy(ident[:], ident_f[:])
        ones_f = persist.tile([P, 1], F32)
        nc.vector.memset(ones_f, 1.0)
        ones_col = persist.tile([P, 1], F32R)
        nc.vector.tensor_copy(ones_col[:], ones_f[:])
        ones_row_f = persist.tile([1, P], F32)
        nc.vector.memset(ones_row_f, 1.0)
        ones_row = persist.tile([1, P], F32R)
        nc.vector.tensor_copy(ones_row[:], ones_row_f[:])
        eps_q = persist.tile([P, 1], F32)
        nc.vector.memset(eps_q, float(HD) * EPS)   # q: rsqrt scale folds 1/sqrt(HD)
        eps_k = persist.tile([P, 1], F32)
        nc.vector.memset(eps_k, EPS)

        aff = {}
        for name, dap in affine_d.items():
            t = persist.tile([P, HD], F32)
            nc.sync.dma_start(t[:], dap[:])
            aff[name] = t

        # ---------------- Phase A: projection + LN + RoPE + transpose --------
        # Pass 1 computes q and k from one sweep over x^T; pass 2 computes v
        # (x^T re-read once). Weight k-tiles stream on the ACT HWDGE queue so
        # their descriptor issue interleaves with the x/out traffic on SP.
        def ln_rope_transpose(pacc, st, is_q, lnp, ropep, psT):
            if stats_bn:
                stats = lnp.tile([P, HPC, 6], F32, tag="stats")
                mv = lnp.tile([P, HPC, 2], F32, tag="mv")
                for h in range(HPC):
                    nc.vector.bn_stats(stats[:, h, :], pacc[:, ts(h, HD)])
                    nc.vector.bn_aggr(mv[:, h, :], stats[:, h, :])
                mean = mv[:, :, 0]
                rstd = lnp.tile([P, HPC], F32, tag="rstd")
                nc.scalar.activation(
                    out=rstd[:], in_=mv[:, :, 1], func=AF.Sqrt,
                    bias=(eps_q if is_q else eps_k)[:],
                    scale=float(HD) if is_q else 1.0)
                nc.vector.reciprocal(out=rstd[:], in_=rstd[:])
            else:
                # sums via DVE reduce, sum-squares via ACT square + reduce
                pview = pacc.rearrange("p (h d) -> p h d", h=HPC)
                sums = lnp.tile([P, HPC], F32, tag="sums")
                nc.vector.reduce_sum(sums[:], pview, axis=mybir.AxisListType.X)
                sq = lnp.tile([P, HPC, HD], F32, tag="sq")
                nc.scalar.activation(out=sq[:], in_=pview, func=AF.Square)
                sumsq = lnp.tile([P, HPC], F32, tag="sumsq")
                nc.vector.reduce_sum(sumsq[:], sq[:], axis=mybir.AxisListType.X)
                mean = lnp.tile([P, HPC], F32, tag="mean")
                nc.vector.tensor_scalar_mul(mean[:], sums[:], 1.0 / HD)
                # hv = HD*var = sumsq - sums^2/HD
                hv = lnp.tile([P, HPC], F32, tag="hv")
                nc.vector.tensor_tensor(hv[:], sums[:], mean[:], ALU.mult)
                nc.vector.tensor_tensor(hv[:], sumsq[:], hv[:], ALU.subtract)
                rstd = lnp.tile([P, HPC], F32, tag="rstd")
                # q: 1/sqrt(HD*(var+eps)) folds the attention 1/sqrt(HD)
                nc.scalar.activation(
                    out=rstd[:], in_=hv[:], func=AF.Sqrt,
                    bias=(eps_q if is_q else eps_k)[:],
                    scale=1.0 if is_q else 1.0 / HD)
                nc.vector.reciprocal(out=rstd[:], in_=rstd[:])
            ln4 = ropep.tile([P, HPC, HD // 2, 2], F32, tag="ln4")
            ln = ln4.rearrange("p h a b -> p h (a b)")
            for h in range(HPC):
                nc.vector.tensor_scalar(
                    out=ln[:, h, :], in0=pacc[:, ts(h, HD)],
                    scalar1=mean[:, h, None] if stats_bn else mean[:, h:h + 1],
                    scalar2=rstd[:, h:h + 1],
                    op0=ALU.subtract, op1=ALU.mult)
            wname, bname = ("qnw", "qnb") if is_q else ("knw", "knb")
            if wname in aff:
                nc.vector.tensor_tensor(
                    ln[:], ln[:],
                    aff[wname][:, None, :].to_broadcast((P, HPC, HD)),
                    ALU.mult)
            if bname in aff:
                nc.vector.tensor_tensor(
                    ln[:], ln[:],
                    aff[bname][:, None, :].to_broadcast((P, HPC, HD)),
                    ALU.add)
            # RoPE (interleaved pairs)
            ce = cos_sb[:, st, None, :].to_broadcast((P, HPC, HD // 2))
            se = sin_sb[:, st, None, :].to_broadcast((P, HPC, HD // 2))
            xe, xo = ln4[:, :, :, 0], ln4[:, :, :, 1]
            rope4 = ropep.tile([P, HPC, HD // 2, 2], F32R, tag="rope4")
            t1 = ropep.tile([P, HPC, HD // 2], F32, tag="t1")
            t2 = ropep.tile([P, HPC, HD // 2], F32, tag="t2")
            eng_a = nc.gpsimd if rope_gp >= 1 else nc.vector
            eng_b = nc.gpsimd if rope_gp >= 2 else nc.vector
            eng_a.tensor_tensor(t1[:], xo, se, ALU.mult)
            eng_a.tensor_tensor(t2[:], xe, se, ALU.mult)
            eng_b.tensor_tensor(rope4[:, :, :, 0], xe, ce, ALU.mult)
            eng_b.tensor_tensor(rope4[:, :, :, 1], xo, ce, ALU.mult)
            nc.vector.tensor_tensor(
                rope4[:, :, :, 0], rope4[:, :, :, 0], t1[:], ALU.subtract)
            nc.vector.tensor_tensor(
                rope4[:, :, :, 1], rope4[:, :, :, 1], t2[:], ALU.add)
            rope = rope4.rearrange("p h a b -> p h (a b)")
            dst = qT if is_q else kTt
            for h in range(HPC):
                ptr = psT.tile([P, P], F32R, tag="ptr")
                nc.tensor.transpose(ptr[:], rope[:, h, :], ident[:])
                if tr_act:
                    ins = nc.scalar.copy(dst[:, h, ts(st, P)], ptr[:])
                else:
                    ins = nc.vector.tensor_copy(dst[:, h, ts(st, P)], ptr[:])
                if chain is not None:
                    chain["last_of_rep"] = ins

        def load_xt(xtp):
            def f(st):
                xt = xtp.tile([P, kt, P], F32R, tag="xt")
                ins = nc.sync.dma_start(xt[:], xt_d[st])
                if chain is not None and "first_of_rep" not in chain:
                    chain["first_of_rep"] = ins
                return [(xt, 0, kt)]
            return f

        def proj_mms(pacc, xparts, wts):
            for xt, k0, k1 in xparts:
                for k in range(k0, k1):
                    nc.tensor.matmul(pacc[:], xt[:, k - k0, :], wts[k][:],
                                     start=(k == 0), stop=(k == kt - 1))

        # v first: SDPA's AV matmuls depend on it, and it frees B to start
        # right as the qk pass drains
        persistV = ctx.enter_context(tc.tile_pool(name="persistV", bufs=1))
        vS = persistV.tile([P, ST, EW], F32R)     # [sk_in_tile, sk_tile, h*HD]
        with ExitStack() as vctx:
            wtpv = vctx.enter_context(tc.tile_pool(name="wtv", bufs=kt + 1))
            xtpv = vctx.enter_context(tc.tile_pool(name="xtv", bufs=xtp_bufs))
            psAv = vctx.enter_context(
                tc.tile_pool(name="psAv", bufs=psA_bufs, space="PSUM"))
            xt_load_v = load_xt(xtpv)
            xt = xt_load_v(0)
            if chain is not None and chain.get("last") is not None:
                from concourse.tile import add_dep_helper
                add_dep_helper(chain["first_of_rep"].ins, chain["last"].ins,
                               sync=True, reason="rep chain")
            wts_v = []
            for k in range(kt):
                wt = wtpv.tile([P, EW], F32R, tag="wt")
                nc.scalar.dma_start(wt[:], wqkvt_d[ts(k, P), ds(2 * EW, EW)])
                wts_v.append(wt)
            nc.sync.dma_start(cos_sb[:], cos_d[:])
            nc.sync.dma_start(sin_sb[:], sin_d[:])
            for st in range(ST):
                if st > 0:
                    xt = xt_load_v(st)
                pacc = psAv.tile([P, EW], F32, tag="pacc")
                proj_mms(pacc, xt, wts_v)
                ins = nc.vector.tensor_copy(vS[:, st, :], pacc[:])
                if chain is not None:
                    chain["last_of_rep"] = ins

        with ExitStack() as actx:
            wtp = actx.enter_context(tc.tile_pool(name="wt", bufs=2 * kt + 1))
            xtp = actx.enter_context(tc.tile_pool(name="xt", bufs=xtp_bufs))
            lnp = actx.enter_context(
                tc.tile_pool(name="ln", bufs=lnrope_bufs))
            ropep = actx.enter_context(
                tc.tile_pool(name="rope", bufs=lnrope_bufs))
            psA = actx.enter_context(
                tc.tile_pool(name="psA", bufs=psA_bufs, space="PSUM"))
            psT = actx.enter_context(
                tc.tile_pool(name="psT", bufs=psT_bufs, space="PSUM"))
            xt_load = load_xt(xtp)

            # pass 2: q & k
            xt = xt_load(0)
            wts_q, wts_k = [], []
            for chunk, wts in ((0, wts_q), (1, wts_k)):
                for k in range(kt):
                    wt = wtp.tile([P, EW], F32R, tag="wt")
                    nc.scalar.dma_start(
                        wt[:], wqkvt_d[ts(k, P), ds(chunk * EW, EW)])
                    wts.append(wt)
            for st in range(ST):
                if st > 0:
                    xt = xt_load(st)
                pacc_q = psA.tile([P, EW], F32, tag="pacc")
                proj_mms(pacc_q, xt, wts_q)
                pacc_k = psA.tile([P, EW], F32, tag="pacc")
                proj_mms(pacc_k, xt, wts_k)
                ln_rope_transpose(pacc_q, st, True, lnp, ropep, psT)
                ln_rope_transpose(pacc_k, st, False, lnp, ropep, psT)

        # wo weights + attention output live in space freed by phase A pools
        persist2 = ctx.enter_context(tc.tile_pool(name="persist2", bufs=1))
        woT = persist2.tile([P, HPC, D], F32R)
        nc.sync.dma_start(woT[:], wot_d.rearrange("(h p) e -> p h e", p=P))

        oT = persist2.tile([P, HPC, S], F32R)  # normalized attention out^T

        # ---------------- Phase B: causal SDPA (transposed layout) -----------
        if "B" not in phases:
            return
        with ExitStack() as bctx:
            expp = bctx.enter_context(tc.tile_pool(name="expp", bufs=expp_bufs))
            rp = bctx.enter_context(tc.tile_pool(name="rp", bufs=2))
            rbp = bctx.enter_context(tc.tile_pool(name="rbp", bufs=2))
            psS = bctx.enter_context(
                tc.tile_pool(name="psS", bufs=psS_bufs, space="PSUM"))
            psO = bctx.enter_context(
                tc.tile_pool(name="psO", bufs=psO_bufs, space="PSUM"))
            psD = bctx.enter_context(
                tc.tile_pool(name="psD", bufs=psD_bufs, space="PSUM"))
            psB = bctx.enter_context(
                tc.tile_pool(name="psB", bufs=psB_bufs, space="PSUM"))

            for c in range(CH):
                tmax = 4 * c + 3  # last causal sk tile for this sq chunk
                for h in range(HPC):
                    po = psO.tile([P, CW], F32, tag="po")
                    pd = psD.tile([1, CW], F32, tag="pd")
                    for t in range(tmax + 1):
                        # causal region: tile t covers sq columns j >= off;
                        # j < off is fully masked, [off, off+P) is the
                        # triangular band, j >= off+P fully valid
                        off = max(0, t * P - c * CW) if trim else 0
                        w = CW - off
                        pscr = psS.tile([P, CW], F32, tag="pscr")
                        nc.tensor.matmul(
                            pscr[:, off:], kTt[:, h, ts(t, P)],
                            qT[:, h, ds(c * CW + off, w)],
                            start=True, stop=True)
                        et = expp.tile([P, CW], F32R, tag="et")
                        nc.scalar.activation(out=et[:, off:], in_=pscr[:, off:],
                                             func=AF.Exp)
                        if (off or t * P == c * CW) if trim else (t >= 4 * c):
                            # zero j<off and mask the band via one select
                            bw = min(off + P, CW)
                            nc.gpsimd.affine_select(
                                out=et[:, :bw], in_=et[:, :bw],
                                pattern=[[1, bw]],
                                channel_multiplier=-1, base=-off,
                                compare_op=ALU.is_ge, fill=0.0)
                        nc.tensor.matmul(po[:, off:], vS[:, t, ts(h, HD)],
                                         et[:, off:],
                                         start=(t == 0), stop=(t == tmax))
                        nc.tensor.matmul(pd[:, off:], ones_col[:], et[:, off:],
                                         start=(t == 0), stop=(t == tmax))
                    r32 = rp.tile([1, CW], F32, tag="r32")
                    nc.vector.reciprocal(out=r32[:], in_=pd[:])
                    rb = rbp.tile([P, CW], F32, tag="rb")
                    rr = rp.tile([1, CW], F32R, tag="rr")
                    nc.vector.tensor_copy(rr[:], r32[:])
                    pb = psB.tile([P, CW], F32, tag="pb")
                    nc.tensor.matmul(pb[:], ones_row[:], rr[:],
                                     start=True, stop=True)
                    nc.vector.tensor_copy(rb[:], pb[:])
                    ins = nc.vector.tensor_tensor(
                        oT[:, h, ds(c * CW, CW)], po[:], rb[:], ALU.mult)
                    if chain is not None:
                        chain["last_of_rep"] = ins

        # ---------------- Phase C: partial output projection ------------------
        if "C" not in phases:
            return
        with ExitStack() as cctx:
            outp = cctx.enter_context(tc.tile_pool(name="outp", bufs=outp_bufs))
            psW = cctx.enter_context(
                tc.tile_pool(name="psW", bufs=psW_bufs, space="PSUM"))
            for st in range(ST):
                for ec in range(CH):
                    pw = psW.tile([P, CW], F32, tag="pw")
                    for h in range(HPC):
                        nc.tensor.matmul(
                            pw[:], oT[:, h, ts(st, P)], woT[:, h, ds(ec * CW, CW)],
                            start=(h == 0), stop=(h == HPC - 1))
                    ot = outp.tile([P, CW], F32, tag="ot")
                    nc.scalar.activation(out=ot[:], in_=pw[:], func=AF.Copy)
                    ins = nc.sync.dma_start(
                        out_d[ts(st, P), ds(ec * CW, CW)], ot[:])
                    if chain is not None:
                        chain["last_of_rep"] = ins


_CACHE = {}


def _get_program(flags):
    if flags not in _CACHE:
        _CACHE[flags] = _build(*flags)
    return _CACHE[flags]


def make_in_maps(x, freqs_cis, wqkv_w, wqkv_b, wo_w, wo_b, qn_w, qn_b,
                 kn_w, kn_b):
    """Host-side sharding: returns (flags, per-core input dicts, wo_b)."""
    x = np.asarray(x, dtype=np.float32)
    freqs_cis = np.asarray(freqs_cis, dtype=np.float32)
    wqkv_w = np.asarray(wqkv_w, dtype=np.float32)
    wqkv_b = np.asarray(wqkv_b, dtype=np.float32)
    wo_w = np.asarray(wo_w, dtype=np.float32)
    qn_w = np.asarray(qn_w, dtype=np.float32)
    qn_b = np.asarray(qn_b, dtype=np.float32)
    kn_w = np.asarray(kn_w, dtype=np.float32)
    kn_b = np.asarray(kn_b, dtype=np.float32)

    with_bias = bool(np.any(wqkv_b != 0.0))
    qn_w_on = bool(np.any(qn_w != 1.0))
    qn_b_on = bool(np.any(qn_b != 0.0))
    kn_w_on = bool(np.any(kn_w != 1.0))
    kn_b_on = bool(np.any(kn_b != 0.0))
    flags = (with_bias, qn_w_on, qn_b_on, kn_w_on, kn_b_on)

    cosf = np.ascontiguousarray(
        freqs_cis[:, :, 0].reshape(ST, P, HD // 2).transpose(1, 0, 2))
    sinf = np.ascontiguousarray(
        freqs_cis[:, :, 1].reshape(ST, P, HD // 2).transpose(1, 0, 2))

    in_maps = []
    for core in range(NC):
        b = core // 4
        g = core % 4
        heads = list(range(g * HPC, (g + 1) * HPC))

        xb = x[:, b, :]                                    # [S, D]
        rows = []
        for base in (0, D, 2 * D):
            for h in heads:
                rows.append(np.arange(base + h * HD, base + (h + 1) * HD))
        rows = np.concatenate(rows)
        wqkvt = np.ascontiguousarray(wqkv_w[rows, :].T)    # [D, 3*EW]
        kt = KT + 1 if with_bias else KT
        if with_bias:
            xb = np.concatenate(
                [xb, np.zeros((S, P), np.float32)], axis=1)
            xb[:, D] = 1.0
            wq_b = np.zeros((P, 3 * EW), np.float32)
            wq_b[0, :] = wqkv_b[rows]
            wqkvt = np.concatenate([wqkvt, wq_b], axis=0)
        # tiled layout [st, p(=d-within-ktile), ko, s]
        xt = np.ascontiguousarray(
            xb.reshape(ST, P, kt, P).transpose(0, 3, 2, 1))

        cols = np.concatenate([np.arange(h * HD, (h + 1) * HD) for h in heads])
        wot = np.ascontiguousarray(wo_w[:, cols].T)        # [EW, D]

        m = {"xt": xt, "wqkvt": wqkvt, "wot": wot, "cosf": cosf, "sinf": sinf}
        if qn_w_on:
            m["qnw"] = np.ascontiguousarray(np.broadcast_to(qn_w, (P, HD)))
        if qn_b_on:
            m["qnb"] = np.ascontiguousarray(
                np.broadcast_to(qn_b * INV_SQRT_HD, (P, HD)))
        if kn_w_on:
            m["knw"] = np.ascontiguousarray(np.broadcast_to(kn_w, (P, HD)))
        if kn_b_on:
            m["knb"] = np.ascontiguousarray(np.broadcast_to(kn_b, (P, HD)))
        in_maps.append(m)

    return flags, in_maps


def kernel(x, freqs_cis, wqkv_w, wqkv_b, wo_w, wo_b, qn_w, qn_b, kn_w, kn_b):
    wo_b = np.asarray(wo_b, dtype=np.float32)
    flags, in_maps = make_in_maps(x, freqs_cis, wqkv_w, wqkv_b, wo_w, wo_b,
                                  qn_w, qn_b, kn_w, kn_b)
    nc = _get_program(flags)
    res = bass_utils.run_bass_kernel_spmd(nc, in_maps, core_ids=list(range(NC)))

    out = np.zeros((S, B, D), dtype=np.float32)
    for core in range(NC):
        out[:, core // 4, :] += res.results[core]["out"]
    out += wo_b[None, None, :]
    return out


# revision 22
# speedup vs baseline: 1.1645x; 1.1645x over previous
"""Trainium2 Bass kernel for nn_SelfAttentionCustom (S=2048, B=2, D=2048, H=16).

Sharding: 8 cores = 2 batches x 4 head-groups (4 heads each). Each core
computes its batch's QKV projection restricted to its heads, QK-layernorm,
RoPE, causal SDPA, and a partial output projection (wo restricted to its
heads' columns). Host sums the 4 partials per batch and adds wo_b.

All matmuls run as float32r (full PE rate at free-dim>=256, ~2e-4 rel err).
Softmax runs without max-subtraction (QK-layernorm bounds logits to ~+-12,
safe in fp32): scoresT tiles [sk,sq] -> exp -> AV matmul in transposed form
(out^T accumulated over sk tiles) and denominator row via ones-vector
matmul; reciprocal is broadcast across partitions with a K=1 matmul.
"""

import sys

sys.path.insert(0, "/opt/trn_rl_repo")

import numpy as np

import concourse.bass as bass
import concourse.tile as tile
from concourse import bacc, mybir
from concourse import bass_utils
from concourse.bass import ds, ts
from concourse.masks import make_identity

F32 = mybir.dt.float32
F32R = mybir.dt.float32r
AF = mybir.ActivationFunctionType
ALU = mybir.AluOpType

S, B, D, H = 2048, 2, 2048, 16
HD = 128          # head dim
NC = 8            # cores
HPC = 4           # heads per core
EPS = 1e-5
P = 128
ST = S // P       # 16 s-tiles
KT = D // P       # 16 contraction k-tiles (projection)
EW = HPC * HD     # 512: per-core width of q/k/v chunk
CH = 4            # sq chunks of 512
CW = S // CH      # 512 chunk width
INV_SQRT_HD = 1.0 / np.sqrt(np.float32(HD))


def _build(with_bias, qn_w_on, qn_b_on, kn_w_on, kn_b_on, **tune):
    """Build the SPMD Bass program (identical on all 8 cores)."""
    nc = bacc.Bacc("TRN2", target_bir_lowering=False, debug=False, num_devices=NC)

    kt = KT + 1 if with_bias else KT
    dk = kt * P

    xt_d = nc.dram_tensor("xt", [ST, P, kt, P], F32R, kind="ExternalInput").ap()
    wqkvt_d = nc.dram_tensor("wqkvt", [dk, 3 * EW], F32R, kind="ExternalInput").ap()
    wot_d = nc.dram_tensor("wot", [EW, D], F32R, kind="ExternalInput").ap()
    cos_d = nc.dram_tensor("cosf", [P, ST, HD // 2], F32, kind="ExternalInput").ap()
    sin_d = nc.dram_tensor("sinf", [P, ST, HD // 2], F32, kind="ExternalInput").ap()
    out_d = nc.dram_tensor("out", [S, D], F32, kind="ExternalOutput").ap()
    affine_d = {}
    for name, on in (("qnw", qn_w_on), ("qnb", qn_b_on),
                     ("knw", kn_w_on), ("knb", kn_b_on)):
        if on:
            affine_d[name] = nc.dram_tensor(
                name, [P, HD], F32, kind="ExternalInput").ap()

    reps = tune.pop("reps", 1)
    with tile.TileContext(nc) as tc:
        chain = {"last": None}
        for _ in range(reps):
            chain.pop("first_of_rep", None)
            _emit(tc, nc, kt, xt_d, wqkvt_d, wot_d, cos_d, sin_d, out_d,
                  affine_d, chain=chain, **tune)
            chain["last"] = chain.get("last_of_rep")
    nc.compile()
    return nc


def _emit(tc, nc, kt, xt_d, wqkvt_d, wot_d, cos_d, sin_d, out_d, affine_d,
          phases="ABC", psA_bufs=4, psT_bufs=4, psS_bufs=3, expp_bufs=6,
          xtp_bufs=2, psW_bufs=4, outp_bufs=6, psO_bufs=2, psD_bufs=2,
          psB_bufs=1, xt_halves=1, dma_bcast=0, tr_act=1, rope_gp=1,
          lnrope_bufs=2, chain=None, stats_bn=1, trim=1, xtpv_bufs=4,
          b_inter=0):
    from contextlib import ExitStack

    with ExitStack() as ctx:
        persist = ctx.enter_context(tc.tile_pool(name="persist", bufs=1))

        # persistent arrays
        qT = persist.tile([P, HPC, S], F32R)      # [d, h, s]
        kTt = persist.tile([P, HPC, S], F32R)
        cos_sb = persist.tile([P, ST, HD // 2], F32)
        sin_sb = persist.tile([P, ST, HD // 2], F32)
        ident_f = persist.tile([P, P], F32)
        make_identity(nc, ident_f)
        ident = persist.tile([P, P], F32R)
        nc.vector.tensor_copy(ident[:], ident_f[:])
        ones_f = persist.tile([P, 1], F32)
        nc.vector.memset(ones_f, 1.0)
        ones_col = persist.tile([P, 1], F32R)
        nc.vector.tensor_copy(ones_col[:], ones_f[:])
        ones_row_f = persist.tile([1, P], F32)
        nc.vector.memset(ones_row_f, 1.0)
        ones_row = persist.tile([1, P], F32R)
        nc.vector.tensor_copy(ones_row[:], ones_row_f[:])
        eps_q = persist.tile([P, 1], F32)
        nc.vector.memset(eps_q, float(HD) * EPS)   # q: rsqrt scale folds 1/sqrt(HD)
        eps_k = persist.tile([P, 1], F32)
        nc.vector.memset(eps_k, EPS)

        aff = {}
        for name, dap in affine_d.items():
            t = persist.tile([P, HD], F32)
            nc.sync.dma_start(t[:], dap[:])
            aff[name] = t

        # ---------------- Phase A: projection + LN + RoPE + transpose --------
        # Pass 1 computes q and k from one sweep over x^T; pass 2 computes v
        # (x^T re-read once). Weight k-tiles stream on the ACT HWDGE queue so
        # their descriptor issue interleaves with the x/out traffic on SP.
        def ln_rope_transpose(pacc, st, is_q, lnp, ropep, psT):
            if stats_bn:
                stats = lnp.tile([P, HPC, 6], F32, tag="stats")
                mv = lnp.tile([P, HPC, 2], F32, tag="mv")
                for h in range(HPC):
                    nc.vector.bn_stats(stats[:, h, :], pacc[:, ts(h, HD)])
                    nc.vector.bn_aggr(mv[:, h, :], stats[:, h, :])
                mean = mv[:, :, 0]
                rstd = lnp.tile([P, HPC], F32, tag="rstd")
                nc.scalar.activation(
                    out=rstd[:], in_=mv[:, :, 1], func=AF.Sqrt,
                    bias=(eps_q if is_q else eps_k)[:],
                    scale=float(HD) if is_q else 1.0)
                nc.vector.reciprocal(out=rstd[:], in_=rstd[:])
            else:
                # sums via DVE reduce, sum-squares via ACT square + reduce
                pview = pacc.rearrange("p (h d) -> p h d", h=HPC)
                sums = lnp.tile([P, HPC], F32, tag="sums")
                nc.vector.reduce_sum(sums[:], pview, axis=mybir.AxisListType.X)
                sq = lnp.tile([P, HPC, HD], F32, tag="sq")
                nc.scalar.activation(out=sq[:], in_=pview, func=AF.Square)
                sumsq = lnp.tile([P, HPC], F32, tag="sumsq")
                nc.vector.reduce_sum(sumsq[:], sq[:], axis=mybir.AxisListType.X)
                mean = lnp.tile([P, HPC], F32, tag="mean")
                nc.vector.tensor_scalar_mul(mean[:], sums[:], 1.0 / HD)
                # hv = HD*var = sumsq - sums^2/HD
                hv = lnp.tile([P, HPC], F32, tag="hv")
                nc.vector.tensor_tensor(hv[:], sums[:], mean[:], ALU.mult)
                nc.vector.tensor_tensor(hv[:], sumsq[:], hv[:], ALU.subtract)
                rstd = lnp.tile([P, HPC], F32, tag="rstd")
                # q: 1/sqrt(HD*(var+eps)) folds the attention 1/sqrt(HD)
                nc.scalar.activation(
                    out=rstd[:], in_=hv[:], func=AF.Sqrt,
                    bias=(eps_q if is_q else eps_k)[:],
                    scale=1.0 if is_q else 1.0 / HD)
                nc.vector.reciprocal(out=rstd[:], in_=rstd[:])
            ln4 = ropep.tile([P, HPC, HD // 2, 2], F32, tag="ln4")
            ln = ln4.rearrange("p h a b -> p h (a b)")
            for h in range(HPC):
                nc.vector.tensor_scalar(
                    out=ln[:, h, :], in0=pacc[:, ts(h, HD)],
                    scalar1=mean[:, h, None] if stats_bn else mean[:, h:h + 1],
                    scalar2=rstd[:, h:h + 1],
                    op0=ALU.subtract, op1=ALU.mult)
            wname, bname = ("qnw", "qnb") if is_q else ("knw", "knb")
            if wname in aff:
                nc.vector.tensor_tensor(
                    ln[:], ln[:],
                    aff[wname][:, None, :].to_broadcast((P, HPC, HD)),
                    ALU.mult)
            if bname in aff:
                nc.vector.tensor_tensor(
                    ln[:], ln[:],
                    aff[bname][:, None, :].to_broadcast((P, HPC, HD)),
                    ALU.add)
            # RoPE (interleaved pairs)
            ce = cos_sb[:, st, None, :].to_broadcast((P, HPC, HD // 2))
            se = sin_sb[:, st, None, :].to_broadcast((P, HPC, HD // 2))
            xe, xo = ln4[:, :, :, 0], ln4[:, :, :, 1]
            rope4 = ropep.tile([P, HPC, HD // 2, 2], F32R, tag="rope4")
            t1 = ropep.tile([P, HPC, HD // 2], F32, tag="t1")
            t2 = ropep.tile([P, HPC, HD // 2], F32, tag="t2")
            eng_a = nc.gpsimd if rope_gp >= 1 else nc.vector
            eng_b = nc.gpsimd if rope_gp >= 2 else nc.vector
            eng_a.tensor_tensor(t1[:], xo, se, ALU.mult)
            eng_a.tensor_tensor(t2[:], xe, se, ALU.mult)
            eng_b.tensor_tensor(rope4[:, :, :, 0], xe, ce, ALU.mult)
            eng_b.tensor_tensor(rope4[:, :, :, 1], xo, ce, ALU.mult)
            nc.vector.tensor_tensor(
                rope4[:, :, :, 0], rope4[:, :, :, 0], t1[:], ALU.subtract)
            nc.vector.tensor_tensor(
                rope4[:, :, :, 1], rope4[:, :, :, 1], t2[:], ALU.add)
            rope = rope4.rearrange("p h a b -> p h (a b)")
            dst = qT if is_q else kTt
            for h in range(HPC):
                ptr = psT.tile([P, P], F32R, tag="ptr")
                nc.tensor.transpose(ptr[:], rope[:, h, :], ident[:])
                if tr_act:
                    ins = nc.scalar.copy(dst[:, h, ts(st, P)], ptr[:])
                else:
                    ins = nc.vector.tensor_copy(dst[:, h, ts(st, P)], ptr[:])
                if chain is not None:
                    chain["last_of_rep"] = ins

        def load_xt(xtp):
            def f(st):
                xt = xtp.tile([P, kt, P], F32R, tag="xt")
                ins = nc.sync.dma_start(xt[:], xt_d[st])
                if chain is not None and "first_of_rep" not in chain:
                    chain["first_of_rep"] = ins
                return [(xt, 0, kt)]
            return f

        def proj_mms(pacc, xparts, wts):
            for xt, k0, k1 in xparts:
                for k in range(k0, k1):
                    nc.tensor.matmul(pacc[:], xt[:, k - k0, :], wts[k][:],
                                     start=(k == 0), stop=(k == kt - 1))

        # v first: SDPA's AV matmuls depend on it, and it frees B to start
        # right as the qk pass drains
        persistV = ctx.enter_context(tc.tile_pool(name="persistV", bufs=1))
        vS = persistV.tile([P, ST, EW], F32R)     # [sk_in_tile, sk_tile, h*HD]
        with ExitStack() as vctx:
            wtpv = vctx.enter_context(tc.tile_pool(name="wtv", bufs=kt + 1))
            xtpv = vctx.enter_context(tc.tile_pool(name="xtv", bufs=xtpv_bufs))
            psAv = vctx.enter_context(
                tc.tile_pool(name="psAv", bufs=psA_bufs, space="PSUM"))
            xt_load_v = load_xt(xtpv)
            xt = xt_load_v(0)
            if chain is not None and chain.get("last") is not None:
                from concourse.tile import add_dep_helper
                add_dep_helper(chain["first_of_rep"].ins, chain["last"].ins,
                               sync=True, reason="rep chain")
            wts_v = []
            for k in range(kt):
                wt = wtpv.tile([P, EW], F32R, tag="wt")
                nc.scalar.dma_start(wt[:], wqkvt_d[ts(k, P), ds(2 * EW, EW)])
                wts_v.append(wt)
            nc.sync.dma_start(cos_sb[:], cos_d[:])
            nc.sync.dma_start(sin_sb[:], sin_d[:])
            for st in range(ST):
                if st > 0:
                    xt = xt_load_v(st)
                pacc = psAv.tile([P, EW], F32, tag="pacc")
                proj_mms(pacc, xt, wts_v)
                ins = nc.vector.tensor_copy(vS[:, st, :], pacc[:])
                if chain is not None:
                    chain["last_of_rep"] = ins

        with ExitStack() as actx:
            wtp = actx.enter_context(tc.tile_pool(name="wt", bufs=2 * kt + 1))
            xtp = actx.enter_context(tc.tile_pool(name="xt", bufs=xtp_bufs))
            lnp = actx.enter_context(
                tc.tile_pool(name="ln", bufs=lnrope_bufs))
            ropep = actx.enter_context(
                tc.tile_pool(name="rope", bufs=lnrope_bufs))
            psA = actx.enter_context(
                tc.tile_pool(name="psA", bufs=psA_bufs, space="PSUM"))
            psT = actx.enter_context(
                tc.tile_pool(name="psT", bufs=psT_bufs, space="PSUM"))
            xt_load = load_xt(xtp)

            # pass 2: q & k
            xt = xt_load(0)
            wts_q, wts_k = [], []
            for chunk, wts in ((0, wts_q), (1, wts_k)):
                for k in range(kt):
                    wt = wtp.tile([P, EW], F32R, tag="wt")
                    nc.scalar.dma_start(
                        wt[:], wqkvt_d[ts(k, P), ds(chunk * EW, EW)])
                    wts.append(wt)
            for st in range(ST):
                if st > 0:
                    xt = xt_load(st)
                pacc_q = psA.tile([P, EW], F32, tag="pacc")
                proj_mms(pacc_q, xt, wts_q)
                pacc_k = psA.tile([P, EW], F32, tag="pacc")
                proj_mms(pacc_k, xt, wts_k)
                ln_rope_transpose(pacc_q, st, True, lnp, ropep, psT)
                ln_rope_transpose(pacc_k, st, False, lnp, ropep, psT)

        # wo weights + attention output live in space freed by phase A pools
        persist2 = ctx.enter_context(tc.tile_pool(name="persist2", bufs=1))
        woT = persist2.tile([P, HPC, D], F32R)
        nc.sync.dma_start(woT[:], wot_d.rearrange("(h p) e -> p h e", p=P))

        oT = persist2.tile([P, HPC, S], F32R)  # normalized attention out^T

        # ---------------- Phase B: causal SDPA (transposed layout) -----------
        if "B" not in phases:
            return
        with ExitStack() as bctx:
            expp = bctx.enter_context(tc.tile_pool(name="expp", bufs=expp_bufs))
            rp = bctx.enter_context(tc.tile_pool(name="rp", bufs=2))
            rbp = bctx.enter_context(tc.tile_pool(name="rbp", bufs=2))
            psS = bctx.enter_context(
                tc.tile_pool(name="psS", bufs=psS_bufs, space="PSUM"))
            psO = bctx.enter_context(
                tc.tile_pool(name="psO", bufs=psO_bufs, space="PSUM"))
            psD = bctx.enter_context(
                tc.tile_pool(name="psD", bufs=psD_bufs, space="PSUM"))
            psB = bctx.enter_context(
                tc.tile_pool(name="psB", bufs=psB_bufs, space="PSUM"))

            if b_inter:
                # head-interleaved: 4 heads advance together per sk tile; the
                # 4 denominator matmuls pack into one PE pass via col-tiling
                for c in range(CH):
                    tmax = 4 * c + 3
                    pos = [psO.tile([P, CW], F32, tag=f"po{h}", name=f"po{h}")
                           for h in range(HPC)]
                    pd = psD.tile([P, CW], F32, tag="pd")
                    for t in range(tmax + 1):
                        off = max(0, t * P - c * CW) if trim else 0
                        w = CW - off
                        ets = []
                        for h in range(HPC):
                            pscr = psS.tile([P, CW], F32, tag="pscr")
                            nc.tensor.matmul(
                                pscr[:, off:], kTt[:, h, ts(t, P)],
                                qT[:, h, ds(c * CW + off, w)],
                                start=True, stop=True)
                            et = expp.tile([P, CW], F32R, tag="et")
                            nc.scalar.activation(out=et[:, off:],
                                                 in_=pscr[:, off:], func=AF.Exp)
                            if (off or t * P == c * CW) if trim else (t >= 4 * c):
                                bw = min(off + P, CW)
                                nc.gpsimd.affine_select(
                                    out=et[:, :bw], in_=et[:, :bw],
                                    pattern=[[1, bw]],
                                    channel_multiplier=-1, base=-off,
                                    compare_op=ALU.is_ge, fill=0.0)
                            nc.tensor.matmul(pos[h][:, off:],
                                             vS[:, t, ts(h, HD)], et[:, off:],
                                             start=(t == 0), stop=(t == tmax))
                            ets.append(et)
                        for h in range(HPC):
                            nc.tensor.matmul(
                                pd[32 * h:32 * h + 1, off:], ones_col[:],
                                ets[h][:, off:],
                                start=(t == 0), stop=(t == tmax),
                                tile_position=(0, 32 * h))
                    r32 = rp.tile([4, CW], F32, tag="r32")
                    nc.vector.reciprocal(
                        out=r32[:],
                        in_=bass.AP(tensor=pd.tensor, offset=pd.offset,
                                    ap=[[32 * pd.ap[0][0], 4]] + pd.ap[1:]))
                    rr = rp.tile([4, CW], F32R, tag="rr")
                    nc.vector.tensor_copy(rr[:], r32[:])
                    for h in range(HPC):
                        pb = psB.tile([P, CW], F32, tag="pb")
                        nc.tensor.matmul(pb[:], ones_row[:], rr[h:h + 1, :],
                                         start=True, stop=True)
                        rb = rbp.tile([P, CW], F32, tag="rb")
                        nc.vector.tensor_copy(rb[:], pb[:])
                        ins = nc.vector.tensor_tensor(
                            oT[:, h, ds(c * CW, CW)], pos[h][:], rb[:],
                            ALU.mult)
                        if chain is not None:
                            chain["last_of_rep"] = ins
            for c in (() if b_inter else range(CH)):
                tmax = 4 * c + 3  # last causal sk tile for this sq chunk
                for h in range(HPC):
                    po = psO.tile([P, CW], F32, tag="po")
                    pd = psD.tile([1, CW], F32, tag="pd")
                    for t in range(tmax + 1):
                        # causal region: tile t covers sq columns j >= off;
                        # j < off is fully masked, [off, off+P) is the
                        # triangular band, j >= off+P fully valid
                        off = max(0, t * P - c * CW) if trim else 0
                        w = CW - off
                        pscr = psS.tile([P, CW], F32, tag="pscr")
                        nc.tensor.matmul(
                            pscr[:, off:], kTt[:, h, ts(t, P)],
                            qT[:, h, ds(c * CW + off, w)],
                            start=True, stop=True)
                        et = expp.tile([P, CW], F32R, tag="et")
                        nc.scalar.activation(out=et[:, off:], in_=pscr[:, off:],
                                             func=AF.Exp)
                        if (off or t * P == c * CW) if trim else (t >= 4 * c):
                            # zero j<off and mask the band via one select
                            bw = min(off + P, CW)
                            nc.gpsimd.affine_select(
                                out=et[:, :bw], in_=et[:, :bw],
                                pattern=[[1, bw]],
                                channel_multiplier=-1, base=-off,
                                compare_op=ALU.is_ge, fill=0.0)
                        nc.tensor.matmul(po[:, off:], vS[:, t, ts(h, HD)],
                                         et[:, off:],
                                         start=(t == 0), stop=(t == tmax))
                        nc.tensor.matmul(pd[:, off:], ones_col[:], et[:, off:],
                                         start=(t == 0), stop=(t == tmax))
                    r32 = rp.tile([1, CW], F32, tag="r32")
                    nc.vector.reciprocal(out=r32[:], in_=pd[:])
                    rb = rbp.tile([P, CW], F32, tag="rb")
                    rr = rp.tile([1, CW], F32R, tag="rr")
                    nc.vector.tensor_copy(rr[:], r32[:])
                    pb = psB.tile([P, CW], F32, tag="pb")
                    nc.tensor.matmul(pb[:], ones_row[:], rr[:],
                                     start=True, stop=True)
                    nc.vector.tensor_copy(rb[:], pb[:])
                    ins = nc.vector.tensor_tensor(
                        oT[:, h, ds(c * CW, CW)], po[:], rb[:], ALU.mult)
                    if chain is not None:
                        chain["last_of_rep"] = ins

        # ---------------- Phase C: partial output projection ------------------
        if "C" not in phases:
            return
        with ExitStack() as cctx:
            outp = cctx.enter_context(tc.tile_pool(name="outp", bufs=outp_bufs))
            psW = cctx.enter_context(
                tc.tile_pool(name="psW", bufs=psW_bufs, space="PSUM"))
            for st in range(ST):
                for ec in range(CH):
                    pw = psW.tile([P, CW], F32, tag="pw")
                    for h in range(HPC):
                        nc.tensor.matmul(
                            pw[:], oT[:, h, ts(st, P)], woT[:, h, ds(ec * CW, CW)],
                            start=(h == 0), stop=(h == HPC - 1))
                    ot = outp.tile([P, CW], F32, tag="ot")
                    nc.scalar.activation(out=ot[:], in_=pw[:], func=AF.Copy)
                    ins = nc.sync.dma_start(
                        out_d[ts(st, P), ds(ec * CW, CW)], ot[:])
                    if chain is not None:
                        chain["last_of_rep"] = ins


_CACHE = {}


def _get_program(flags):
    if flags not in _CACHE:
        _CACHE[flags] = _build(*flags)
    return _CACHE[flags]


def make_in_maps(x, freqs_cis, wqkv_w, wqkv_b, wo_w, wo_b, qn_w, qn_b,
                 kn_w, kn_b):
    """Host-side sharding: returns (flags, per-core input dicts, wo_b)."""
    x = np.asarray(x, dtype=np.float32)
    freqs_cis = np.asarray(freqs_cis, dtype=np.float32)
    wqkv_w = np.asarray(wqkv_w, dtype=np.float32)
    wqkv_b = np.asarray(wqkv_b, dtype=np.float32)
    wo_w = np.asarray(wo_w, dtype=np.float32)
    qn_w = np.asarray(qn_w, dtype=np.float32)
    qn_b = np.asarray(qn_b, dtype=np.float32)
    kn_w = np.asarray(kn_w, dtype=np.float32)
    kn_b = np.asarray(kn_b, dtype=np.float32)

    with_bias = bool(np.any(wqkv_b != 0.0))
    qn_w_on = bool(np.any(qn_w != 1.0))
    qn_b_on = bool(np.any(qn_b != 0.0))
    kn_w_on = bool(np.any(kn_w != 1.0))
    kn_b_on = bool(np.any(kn_b != 0.0))
    flags = (with_bias, qn_w_on, qn_b_on, kn_w_on, kn_b_on)

    cosf = np.ascontiguousarray(
        freqs_cis[:, :, 0].reshape(ST, P, HD // 2).transpose(1, 0, 2))
    sinf = np.ascontiguousarray(
        freqs_cis[:, :, 1].reshape(ST, P, HD // 2).transpose(1, 0, 2))

    in_maps = []
    for core in range(NC):
        b = core // 4
        g = core % 4
        heads = list(range(g * HPC, (g + 1) * HPC))

        xb = x[:, b, :]                                    # [S, D]
        rows = []
        for base in (0, D, 2 * D):
            for h in heads:
                rows.append(np.arange(base + h * HD, base + (h + 1) * HD))
        rows = np.concatenate(rows)
        wqkvt = np.ascontiguousarray(wqkv_w[rows, :].T)    # [D, 3*EW]
        kt = KT + 1 if with_bias else KT
        if with_bias:
            xb = np.concatenate(
                [xb, np.zeros((S, P), np.float32)], axis=1)
            xb[:, D] = 1.0
            wq_b = np.zeros((P, 3 * EW), np.float32)
            wq_b[0, :] = wqkv_b[rows]
            wqkvt = np.concatenate([wqkvt, wq_b], axis=0)
        # tiled layout [st, p(=d-within-ktile), ko, s]
        xt = np.ascontiguousarray(
            xb.reshape(ST, P, kt, P).transpose(0, 3, 2, 1))

        cols = np.concatenate([np.arange(h * HD, (h + 1) * HD) for h in heads])
        wot = np.ascontiguousarray(wo_w[:, cols].T)        # [EW, D]

        m = {"xt": xt, "wqkvt": wqkvt, "wot": wot, "cosf": cosf, "sinf": sinf}
        if qn_w_on:
            m["qnw"] = np.ascontiguousarray(np.broadcast_to(qn_w, (P, HD)))
        if qn_b_on:
            m["qnb"] = np.ascontiguousarray(
                np.broadcast_to(qn_b * INV_SQRT_HD, (P, HD)))
        if kn_w_on:
            m["knw"] = np.ascontiguousarray(np.broadcast_to(kn_w, (P, HD)))
        if kn_b_on:
            m["knb"] = np.ascontiguousarray(np.broadcast_to(kn_b, (P, HD)))
        in_maps.append(m)

    return flags, in_maps


def kernel(x, freqs_cis, wqkv_w, wqkv_b, wo_w, wo_b, qn_w, qn_b, kn_w, kn_b):
    wo_b = np.asarray(wo_b, dtype=np.float32)
    flags, in_maps = make_in_maps(x, freqs_cis, wqkv_w, wqkv_b, wo_w, wo_b,
                                  qn_w, qn_b, kn_w, kn_b)
    nc = _get_program(flags)
    res = bass_utils.run_bass_kernel_spmd(nc, in_maps, core_ids=list(range(NC)))

    out = np.zeros((S, B, D), dtype=np.float32)
    for core in range(NC):
        out[:, core // 4, :] += res.results[core]["out"]
    out += wo_b[None, None, :]
    return out


# revision 23
# speedup vs baseline: 1.1785x; 1.0120x over previous
"""Trainium2 Bass kernel for nn_SelfAttentionCustom (S=2048, B=2, D=2048, H=16).

Sharding: 8 cores = 2 batches x 4 head-groups (4 heads each). Each core
computes its batch's QKV projection restricted to its heads, QK-layernorm,
RoPE, causal SDPA, and a partial output projection (wo restricted to its
heads' columns). Host sums the 4 partials per batch and adds wo_b.

All matmuls run as float32r (full PE rate at free-dim>=256, ~2e-4 rel err).
Softmax runs without max-subtraction (QK-layernorm bounds logits to ~+-12,
safe in fp32): scoresT tiles [sk,sq] -> exp -> AV matmul in transposed form
(out^T accumulated over sk tiles) and denominator row via ones-vector
matmul; reciprocal is broadcast across partitions with a K=1 matmul.
"""

import sys

sys.path.insert(0, "/opt/trn_rl_repo")

import numpy as np

import concourse.bass as bass
import concourse.tile as tile
from concourse import bacc, mybir
from concourse import bass_utils
from concourse.bass import ds, ts
from concourse.masks import make_identity

F32 = mybir.dt.float32
F32R = mybir.dt.float32r
AF = mybir.ActivationFunctionType
ALU = mybir.AluOpType

S, B, D, H = 2048, 2, 2048, 16
HD = 128          # head dim
NC = 8            # cores
HPC = 4           # heads per core
EPS = 1e-5
P = 128
ST = S // P       # 16 s-tiles
KT = D // P       # 16 contraction k-tiles (projection)
EW = HPC * HD     # 512: per-core width of q/k/v chunk
CH = 4            # sq chunks of 512
CW = S // CH      # 512 chunk width
INV_SQRT_HD = 1.0 / np.sqrt(np.float32(HD))


def _build(with_bias, qn_w_on, qn_b_on, kn_w_on, kn_b_on, **tune):
    """Build the SPMD Bass program (identical on all 8 cores)."""
    nc = bacc.Bacc("TRN2", target_bir_lowering=False, debug=False, num_devices=NC)

    kt = KT + 1 if with_bias else KT
    dk = kt * P

    xt_d = nc.dram_tensor("xt", [ST, P, kt, P], F32R, kind="ExternalInput").ap()
    wqkvt_d = nc.dram_tensor("wqkvt", [dk, 3 * EW], F32R, kind="ExternalInput").ap()
    wot_d = nc.dram_tensor("wot", [EW, D], F32R, kind="ExternalInput").ap()
    cos_d = nc.dram_tensor("cosf", [P, ST, HD // 2], F32, kind="ExternalInput").ap()
    sin_d = nc.dram_tensor("sinf", [P, ST, HD // 2], F32, kind="ExternalInput").ap()
    out_d = nc.dram_tensor("out", [S, D], F32, kind="ExternalOutput").ap()
    affine_d = {}
    for name, on in (("qnw", qn_w_on), ("qnb", qn_b_on),
                     ("knw", kn_w_on), ("knb", kn_b_on)):
        if on:
            affine_d[name] = nc.dram_tensor(
                name, [P, HD], F32, kind="ExternalInput").ap()

    reps = tune.pop("reps", 1)
    with tile.TileContext(nc) as tc:
        chain = {"last": None}
        for _ in range(reps):
            chain.pop("first_of_rep", None)
            _emit(tc, nc, kt, xt_d, wqkvt_d, wot_d, cos_d, sin_d, out_d,
                  affine_d, chain=chain, **tune)
            chain["last"] = chain.get("last_of_rep")
    nc.compile()
    return nc


def _emit(tc, nc, kt, xt_d, wqkvt_d, wot_d, cos_d, sin_d, out_d, affine_d,
          phases="ABC", psA_bufs=4, psT_bufs=4, psS_bufs=4, expp_bufs=6,
          xtp_bufs=2, psW_bufs=4, outp_bufs=6, psO_bufs=2, psD_bufs=1,
          psB_bufs=1, xt_halves=1, dma_bcast=0, tr_act=1, rope_gp=1,
          lnrope_bufs=2, chain=None, stats_bn=1, trim=1, xtpv_bufs=4,
          b_inter=0):
    from contextlib import ExitStack

    with ExitStack() as ctx:
        persist = ctx.enter_context(tc.tile_pool(name="persist", bufs=1))

        # persistent arrays
        qT = persist.tile([P, HPC, S], F32R)      # [d, h, s]
        kTt = persist.tile([P, HPC, S], F32R)
        cos_sb = persist.tile([P, ST, HD // 2], F32)
        sin_sb = persist.tile([P, ST, HD // 2], F32)
        ident_f = persist.tile([P, P], F32)
        make_identity(nc, ident_f)
        ident = persist.tile([P, P], F32R)
        nc.vector.tensor_copy(ident[:], ident_f[:])
        ones_f = persist.tile([P, 1], F32)
        nc.vector.memset(ones_f, 1.0)
        ones_col = persist.tile([P, 1], F32R)
        nc.vector.tensor_copy(ones_col[:], ones_f[:])
        ones_row_f = persist.tile([1, P], F32)
        nc.vector.memset(ones_row_f, 1.0)
        ones_row = persist.tile([1, P], F32R)
        nc.vector.tensor_copy(ones_row[:], ones_row_f[:])
        eps_q = persist.tile([P, 1], F32)
        nc.vector.memset(eps_q, float(HD) * EPS)   # q: rsqrt scale folds 1/sqrt(HD)
        eps_k = persist.tile([P, 1], F32)
        nc.vector.memset(eps_k, EPS)

        aff = {}
        for name, dap in affine_d.items():
            t = persist.tile([P, HD], F32)
            nc.sync.dma_start(t[:], dap[:])
            aff[name] = t

        # ---------------- Phase A: projection + LN + RoPE + transpose --------
        # Pass 1 computes q and k from one sweep over x^T; pass 2 computes v
        # (x^T re-read once). Weight k-tiles stream on the ACT HWDGE queue so
        # their descriptor issue interleaves with the x/out traffic on SP.
        def ln_rope_transpose(pacc, st, is_q, lnp, ropep, psT):
            if stats_bn:
                stats = lnp.tile([P, HPC, 6], F32, tag="stats")
                mv = lnp.tile([P, HPC, 2], F32, tag="mv")
                for h in range(HPC):
                    nc.vector.bn_stats(stats[:, h, :], pacc[:, ts(h, HD)])
                    nc.vector.bn_aggr(mv[:, h, :], stats[:, h, :])
                mean = mv[:, :, 0]
                rstd = lnp.tile([P, HPC], F32, tag="rstd")
                nc.scalar.activation(
                    out=rstd[:], in_=mv[:, :, 1], func=AF.Sqrt,
                    bias=(eps_q if is_q else eps_k)[:],
                    scale=float(HD) if is_q else 1.0)
                nc.vector.reciprocal(out=rstd[:], in_=rstd[:])
            else:
                # sums via DVE reduce, sum-squares via ACT square + reduce
                pview = pacc.rearrange("p (h d) -> p h d", h=HPC)
                sums = lnp.tile([P, HPC], F32, tag="sums")
                nc.vector.reduce_sum(sums[:], pview, axis=mybir.AxisListType.X)
                sq = lnp.tile([P, HPC, HD], F32, tag="sq")
                nc.scalar.activation(out=sq[:], in_=pview, func=AF.Square)
                sumsq = lnp.tile([P, HPC], F32, tag="sumsq")
                nc.vector.reduce_sum(sumsq[:], sq[:], axis=mybir.AxisListType.X)
                mean = lnp.tile([P, HPC], F32, tag="mean")
                nc.vector.tensor_scalar_mul(mean[:], sums[:], 1.0 / HD)
                # hv = HD*var = sumsq - sums^2/HD
                hv = lnp.tile([P, HPC], F32, tag="hv")
                nc.vector.tensor_tensor(hv[:], sums[:], mean[:], ALU.mult)
                nc.vector.tensor_tensor(hv[:], sumsq[:], hv[:], ALU.subtract)
                rstd = lnp.tile([P, HPC], F32, tag="rstd")
                # q: 1/sqrt(HD*(var+eps)) folds the attention 1/sqrt(HD)
                nc.scalar.activation(
                    out=rstd[:], in_=hv[:], func=AF.Sqrt,
                    bias=(eps_q if is_q else eps_k)[:],
                    scale=1.0 if is_q else 1.0 / HD)
                nc.vector.reciprocal(out=rstd[:], in_=rstd[:])
            ln4 = ropep.tile([P, HPC, HD // 2, 2], F32, tag="ln4")
            ln = ln4.rearrange("p h a b -> p h (a b)")
            for h in range(HPC):
                nc.vector.tensor_scalar(
                    out=ln[:, h, :], in0=pacc[:, ts(h, HD)],
                    scalar1=mean[:, h, None] if stats_bn else mean[:, h:h + 1],
                    scalar2=rstd[:, h:h + 1],
                    op0=ALU.subtract, op1=ALU.mult)
            wname, bname = ("qnw", "qnb") if is_q else ("knw", "knb")
            if wname in aff:
                nc.vector.tensor_tensor(
                    ln[:], ln[:],
                    aff[wname][:, None, :].to_broadcast((P, HPC, HD)),
                    ALU.mult)
            if bname in aff:
                nc.vector.tensor_tensor(
                    ln[:], ln[:],
                    aff[bname][:, None, :].to_broadcast((P, HPC, HD)),
                    ALU.add)
            # RoPE (interleaved pairs)
            ce = cos_sb[:, st, None, :].to_broadcast((P, HPC, HD // 2))
            se = sin_sb[:, st, None, :].to_broadcast((P, HPC, HD // 2))
            xe, xo = ln4[:, :, :, 0], ln4[:, :, :, 1]
            rope4 = ropep.tile([P, HPC, HD // 2, 2], F32R, tag="rope4")
            t1 = ropep.tile([P, HPC, HD // 2], F32, tag="t1")
            t2 = ropep.tile([P, HPC, HD // 2], F32, tag="t2")
            eng_a = nc.gpsimd if rope_gp >= 1 else nc.vector
            eng_b = nc.gpsimd if rope_gp >= 2 else nc.vector
            eng_a.tensor_tensor(t1[:], xo, se, ALU.mult)
            eng_a.tensor_tensor(t2[:], xe, se, ALU.mult)
            eng_b.tensor_tensor(rope4[:, :, :, 0], xe, ce, ALU.mult)
            eng_b.tensor_tensor(rope4[:, :, :, 1], xo, ce, ALU.mult)
            nc.vector.tensor_tensor(
                rope4[:, :, :, 0], rope4[:, :, :, 0], t1[:], ALU.subtract)
            nc.vector.tensor_tensor(
                rope4[:, :, :, 1], rope4[:, :, :, 1], t2[:], ALU.add)
            rope = rope4.rearrange("p h a b -> p h (a b)")
            dst = qT if is_q else kTt
            for h in range(HPC):
                ptr = psT.tile([P, P], F32R, tag="ptr")
                nc.tensor.transpose(ptr[:], rope[:, h, :], ident[:])
                if tr_act:
                    ins = nc.scalar.copy(dst[:, h, ts(st, P)], ptr[:])
                else:
                    ins = nc.vector.tensor_copy(dst[:, h, ts(st, P)], ptr[:])
                if chain is not None:
                    chain["last_of_rep"] = ins

        def load_xt(xtp):
            def f(st):
                xt = xtp.tile([P, kt, P], F32R, tag="xt")
                ins = nc.sync.dma_start(xt[:], xt_d[st])
                if chain is not None and "first_of_rep" not in chain:
                    chain["first_of_rep"] = ins
                return [(xt, 0, kt)]
            return f

        def proj_mms(pacc, xparts, wts):
            for xt, k0, k1 in xparts:
                for k in range(k0, k1):
                    nc.tensor.matmul(pacc[:], xt[:, k - k0, :], wts[k][:],
                                     start=(k == 0), stop=(k == kt - 1))

        # v first: SDPA's AV matmuls depend on it, and it frees B to start
        # right as the qk pass drains
        persistV = ctx.enter_context(tc.tile_pool(name="persistV", bufs=1))
        vS = persistV.tile([P, ST, EW], F32R)     # [sk_in_tile, sk_tile, h*HD]
        with ExitStack() as vctx:
            wtpv = vctx.enter_context(tc.tile_pool(name="wtv", bufs=kt + 1))
            xtpv = vctx.enter_context(tc.tile_pool(name="xtv", bufs=xtpv_bufs))
            psAv = vctx.enter_context(
                tc.tile_pool(name="psAv", bufs=psA_bufs, space="PSUM"))
            xt_load_v = load_xt(xtpv)
            xt = xt_load_v(0)
            if chain is not None and chain.get("last") is not None:
                from concourse.tile import add_dep_helper
                add_dep_helper(chain["first_of_rep"].ins, chain["last"].ins,
                               sync=True, reason="rep chain")
            wts_v = []
            for k in range(kt):
                wt = wtpv.tile([P, EW], F32R, tag="wt")
                nc.scalar.dma_start(wt[:], wqkvt_d[ts(k, P), ds(2 * EW, EW)])
                wts_v.append(wt)
            nc.sync.dma_start(cos_sb[:], cos_d[:])
            nc.sync.dma_start(sin_sb[:], sin_d[:])
            for st in range(ST):
                if st > 0:
                    xt = xt_load_v(st)
                pacc = psAv.tile([P, EW], F32, tag="pacc")
                proj_mms(pacc, xt, wts_v)
                ins = nc.vector.tensor_copy(vS[:, st, :], pacc[:])
                if chain is not None:
                    chain["last_of_rep"] = ins

        with ExitStack() as actx:
            wtp = actx.enter_context(tc.tile_pool(name="wt", bufs=2 * kt + 1))
            xtp = actx.enter_context(tc.tile_pool(name="xt", bufs=xtp_bufs))
            lnp = actx.enter_context(
                tc.tile_pool(name="ln", bufs=lnrope_bufs))
            ropep = actx.enter_context(
                tc.tile_pool(name="rope", bufs=lnrope_bufs))
            psA = actx.enter_context(
                tc.tile_pool(name="psA", bufs=psA_bufs, space="PSUM"))
            psT = actx.enter_context(
                tc.tile_pool(name="psT", bufs=psT_bufs, space="PSUM"))
            xt_load = load_xt(xtp)

            # pass 2: q & k
            xt = xt_load(0)
            wts_q, wts_k = [], []
            for chunk, wts in ((0, wts_q), (1, wts_k)):
                for k in range(kt):
                    wt = wtp.tile([P, EW], F32R, tag="wt")
                    nc.scalar.dma_start(
                        wt[:], wqkvt_d[ts(k, P), ds(chunk * EW, EW)])
                    wts.append(wt)
            for st in range(ST):
                if st > 0:
                    xt = xt_load(st)
                pacc_q = psA.tile([P, EW], F32, tag="pacc")
                proj_mms(pacc_q, xt, wts_q)
                pacc_k = psA.tile([P, EW], F32, tag="pacc")
                proj_mms(pacc_k, xt, wts_k)
                ln_rope_transpose(pacc_q, st, True, lnp, ropep, psT)
                ln_rope_transpose(pacc_k, st, False, lnp, ropep, psT)

        # wo weights + attention output live in space freed by phase A pools
        persist2 = ctx.enter_context(tc.tile_pool(name="persist2", bufs=1))
        woT = persist2.tile([P, HPC, D], F32R)
        nc.sync.dma_start(woT[:], wot_d.rearrange("(h p) e -> p h e", p=P))

        oT = persist2.tile([P, HPC, S], F32R)  # normalized attention out^T

        # ---------------- Phase B: causal SDPA (transposed layout) -----------
        if "B" not in phases:
            return
        with ExitStack() as bctx:
            expp = bctx.enter_context(tc.tile_pool(name="expp", bufs=expp_bufs))
            rp = bctx.enter_context(tc.tile_pool(name="rp", bufs=2))
            rbp = bctx.enter_context(tc.tile_pool(name="rbp", bufs=2))
            psS = bctx.enter_context(
                tc.tile_pool(name="psS", bufs=psS_bufs, space="PSUM"))
            psO = bctx.enter_context(
                tc.tile_pool(name="psO", bufs=psO_bufs, space="PSUM"))
            psD = bctx.enter_context(
                tc.tile_pool(name="psD", bufs=psD_bufs, space="PSUM"))
            psB = bctx.enter_context(
                tc.tile_pool(name="psB", bufs=psB_bufs, space="PSUM"))

            if b_inter:
                # head-interleaved: 4 heads advance together per sk tile; the
                # 4 denominator matmuls pack into one PE pass via col-tiling
                for c in range(CH):
                    tmax = 4 * c + 3
                    pos = [psO.tile([P, CW], F32, tag=f"po{h}", name=f"po{h}")
                           for h in range(HPC)]
                    pd = psD.tile([P, CW], F32, tag="pd")
                    for t in range(tmax + 1):
                        off = max(0, t * P - c * CW) if trim else 0
                        w = CW - off
                        ets = []
                        for h in range(HPC):
                            pscr = psS.tile([P, CW], F32, tag="pscr")
                            nc.tensor.matmul(
                                pscr[:, off:], kTt[:, h, ts(t, P)],
                                qT[:, h, ds(c * CW + off, w)],
                                start=True, stop=True)
                            et = expp.tile([P, CW], F32R, tag="et")
                            nc.scalar.activation(out=et[:, off:],
                                                 in_=pscr[:, off:], func=AF.Exp)
                            if (off or t * P == c * CW) if trim else (t >= 4 * c):
                                bw = min(off + P, CW)
                                nc.gpsimd.affine_select(
                                    out=et[:, :bw], in_=et[:, :bw],
                                    pattern=[[1, bw]],
                                    channel_multiplier=-1, base=-off,
                                    compare_op=ALU.is_ge, fill=0.0)
                            nc.tensor.matmul(pos[h][:, off:],
                                             vS[:, t, ts(h, HD)], et[:, off:],
                                             start=(t == 0), stop=(t == tmax))
                            ets.append(et)
                        for h in range(HPC):
                            nc.tensor.matmul(
                                pd[32 * h:32 * h + 1, off:], ones_col[:],
                                ets[h][:, off:],
                                start=(t == 0), stop=(t == tmax),
                                tile_position=(0, 32 * h))
                    r32 = rp.tile([4, CW], F32, tag="r32")
                    nc.vector.reciprocal(
                        out=r32[:],
                        in_=bass.AP(tensor=pd.tensor, offset=pd.offset,
                                    ap=[[32 * pd.ap[0][0], 4]] + pd.ap[1:]))
                    rr = rp.tile([4, CW], F32R, tag="rr")
                    nc.vector.tensor_copy(rr[:], r32[:])
                    for h in range(HPC):
                        pb = psB.tile([P, CW], F32, tag="pb")
                        nc.tensor.matmul(pb[:], ones_row[:], rr[h:h + 1, :],
                                         start=True, stop=True)
                        rb = rbp.tile([P, CW], F32, tag="rb")
                        nc.vector.tensor_copy(rb[:], pb[:])
                        ins = nc.vector.tensor_tensor(
                            oT[:, h, ds(c * CW, CW)], pos[h][:], rb[:],
                            ALU.mult)
                        if chain is not None:
                            chain["last_of_rep"] = ins
            for c in (() if b_inter else range(CH)):
                tmax = 4 * c + 3  # last causal sk tile for this sq chunk
                for h in range(HPC):
                    po = psO.tile([P, CW], F32, tag="po")
                    pd = psD.tile([1, CW], F32, tag="pd")
                    for t in range(tmax + 1):
                        # causal region: tile t covers sq columns j >= off;
                        # j < off is fully masked, [off, off+P) is the
                        # triangular band, j >= off+P fully valid
                        off = max(0, t * P - c * CW) if trim else 0
                        w = CW - off
                        pscr = psS.tile([P, CW], F32, tag="pscr")
                        nc.tensor.matmul(
                            pscr[:, off:], kTt[:, h, ts(t, P)],
                            qT[:, h, ds(c * CW + off, w)],
                            start=True, stop=True)
                        et = expp.tile([P, CW], F32R, tag="et")
                        nc.scalar.activation(out=et[:, off:], in_=pscr[:, off:],
                                             func=AF.Exp)
                        if (off or t * P == c * CW) if trim else (t >= 4 * c):
                            # zero j<off and mask the band via one select
                            bw = min(off + P, CW)
                            nc.gpsimd.affine_select(
                                out=et[:, :bw], in_=et[:, :bw],
                                pattern=[[1, bw]],
                                channel_multiplier=-1, base=-off,
                                compare_op=ALU.is_ge, fill=0.0)
                        nc.tensor.matmul(po[:, off:], vS[:, t, ts(h, HD)],
                                         et[:, off:],
                                         start=(t == 0), stop=(t == tmax))
                        nc.tensor.matmul(pd[:, off:], ones_col[:], et[:, off:],
                                         start=(t == 0), stop=(t == tmax))
                    r32 = rp.tile([1, CW], F32, tag="r32")
                    nc.vector.reciprocal(out=r32[:], in_=pd[:])
                    rb = rbp.tile([P, CW], F32, tag="rb")
                    rr = rp.tile([1, CW], F32R, tag="rr")
                    nc.vector.tensor_copy(rr[:], r32[:])
                    pb = psB.tile([P, CW], F32, tag="pb")
                    nc.tensor.matmul(pb[:], ones_row[:], rr[:],
                                     start=True, stop=True)
                    nc.vector.tensor_copy(rb[:], pb[:])
                    ins = nc.vector.tensor_tensor(
                        oT[:, h, ds(c * CW, CW)], po[:], rb[:], ALU.mult)
                    if chain is not None:
                        chain["last_of_rep"] = ins

        # ---------------- Phase C: partial output projection ------------------
        if "C" not in phases:
            return
        with ExitStack() as cctx:
            outp = cctx.enter_context(tc.tile_pool(name="outp", bufs=outp_bufs))
            psW = cctx.enter_context(
                tc.tile_pool(name="psW", bufs=psW_bufs, space="PSUM"))
            for st in range(ST):
                for ec in range(CH):
                    pw = psW.tile([P, CW], F32, tag="pw")
                    for h in range(HPC):
                        nc.tensor.matmul(
                            pw[:], oT[:, h, ts(st, P)], woT[:, h, ds(ec * CW, CW)],
                            start=(h == 0), stop=(h == HPC - 1))
                    ot = outp.tile([P, CW], F32, tag="ot")
                    nc.scalar.activation(out=ot[:], in_=pw[:], func=AF.Copy)
                    ins = nc.sync.dma_start(
                        out_d[ts(st, P), ds(ec * CW, CW)], ot[:])
                    if chain is not None:
                        chain["last_of_rep"] = ins


_CACHE = {}


def _get_program(flags):
    if flags not in _CACHE:
        _CACHE[flags] = _build(*flags)
    return _CACHE[flags]


def make_in_maps(x, freqs_cis, wqkv_w, wqkv_b, wo_w, wo_b, qn_w, qn_b,
                 kn_w, kn_b):
    """Host-side sharding: returns (flags, per-core input dicts, wo_b)."""
    x = np.asarray(x, dtype=np.float32)
    freqs_cis = np.asarray(freqs_cis, dtype=np.float32)
    wqkv_w = np.asarray(wqkv_w, dtype=np.float32)
    wqkv_b = np.asarray(wqkv_b, dtype=np.float32)
    wo_w = np.asarray(wo_w, dtype=np.float32)
    qn_w = np.asarray(qn_w, dtype=np.float32)
    qn_b = np.asarray(qn_b, dtype=np.float32)
    kn_w = np.asarray(kn_w, dtype=np.float32)
    kn_b = np.asarray(kn_b, dtype=np.float32)

    with_bias = bool(np.any(wqkv_b != 0.0))
    qn_w_on = bool(np.any(qn_w != 1.0))
    qn_b_on = bool(np.any(qn_b != 0.0))
    kn_w_on = bool(np.any(kn_w != 1.0))
    kn_b_on = bool(np.any(kn_b != 0.0))
    flags = (with_bias, qn_w_on, qn_b_on, kn_w_on, kn_b_on)

    cosf = np.ascontiguousarray(
        freqs_cis[:, :, 0].reshape(ST, P, HD // 2).transpose(1, 0, 2))
    sinf = np.ascontiguousarray(
        freqs_cis[:, :, 1].reshape(ST, P, HD // 2).transpose(1, 0, 2))

    in_maps = []
    for core in range(NC):
        b = core // 4
        g = core % 4
        heads = list(range(g * HPC, (g + 1) * HPC))

        xb = x[:, b, :]                                    # [S, D]
        rows = []
        for base in (0, D, 2 * D):
            for h in heads:
                rows.append(np.arange(base + h * HD, base + (h + 1) * HD))
        rows = np.concatenate(rows)
        wqkvt = np.ascontiguousarray(wqkv_w[rows, :].T)    # [D, 3*EW]
        kt = KT + 1 if with_bias else KT
        if with_bias:
            xb = np.concatenate(
                [xb, np.zeros((S, P), np.float32)], axis=1)
            xb[:, D] = 1.0
            wq_b = np.zeros((P, 3 * EW), np.float32)
            wq_b[0, :] = wqkv_b[rows]
            wqkvt = np.concatenate([wqkvt, wq_b], axis=0)
        # tiled layout [st, p(=d-within-ktile), ko, s]
        xt = np.ascontiguousarray(
            xb.reshape(ST, P, kt, P).transpose(0, 3, 2, 1))

        cols = np.concatenate([np.arange(h * HD, (h + 1) * HD) for h in heads])
        wot = np.ascontiguousarray(wo_w[:, cols].T)        # [EW, D]

        m = {"xt": xt, "wqkvt": wqkvt, "wot": wot, "cosf": cosf, "sinf": sinf}
        if qn_w_on:
            m["qnw"] = np.ascontiguousarray(np.broadcast_to(qn_w, (P, HD)))
        if qn_b_on:
            m["qnb"] = np.ascontiguousarray(
                np.broadcast_to(qn_b * INV_SQRT_HD, (P, HD)))
        if kn_w_on:
            m["knw"] = np.ascontiguousarray(np.broadcast_to(kn_w, (P, HD)))
        if kn_b_on:
            m["knb"] = np.ascontiguousarray(np.broadcast_to(kn_b, (P, HD)))
        in_maps.append(m)

    return flags, in_maps


def kernel(x, freqs_cis, wqkv_w, wqkv_b, wo_w, wo_b, qn_w, qn_b, kn_w, kn_b):
    wo_b = np.asarray(wo_b, dtype=np.float32)
    flags, in_maps = make_in_maps(x, freqs_cis, wqkv_w, wqkv_b, wo_w, wo_b,
                                  qn_w, qn_b, kn_w, kn_b)
    nc = _get_program(flags)
    res = bass_utils.run_bass_kernel_spmd(nc, in_maps, core_ids=list(range(NC)))

    out = np.zeros((S, B, D), dtype=np.float32)
    for core in range(NC):
        out[:, core // 4, :] += res.results[core]["out"]
    out += wo_b[None, None, :]
    return out
